# revision 17
# baseline (speedup 1.0000x reference)
"""Trainium2 Bass kernel for a Keras-style GRU (reset_after=True) + Dense(1) head.

Reference computation (per batch row):
    x_proj = x @ kernel + bias_i                      # [T, 3H]
    per step t:  hp = h @ rkernel + bias_r            # [3H]
        z  = sigmoid(xp[:H]      + hp[:H])
        r  = sigmoid(xp[H:2H]    + hp[H:2H])
        hh = tanh   (xp[2H:]     + r * hp[2H:])
        h  = z * h + (1 - z) * hh
    out = h_last @ dense_w + dense_b                  # [1]

Strategy (8 NeuronCores, data-parallel over batch, 64 rows/core):
  - The per-step serial cycle (rec matmuls -> sigmoid -> t1 -> t2 -> tanh ->
    blend -> next rec) is latency-bound by ~5 cross-engine semaphore hops of
    ~300ns each, so a single dependence chain can't go below ~3.5us/step.
    Instead each core runs TWO independent GRU chains (batch rows split
    32+32), phase-shifted half a cycle: every ~3.5us cycle completes one step
    of BOTH chains => ~1.75us/step effective.
  - All recurrence matmuls are fp8(e4m3) DoubleRow (two K-chunks per
    instruction, half cycles/col): h is quantized to fp8 as a matmul input
    only; the carried state stays bf16 (numerically validated: ~1e-2).
  - x_proj for all three gates is a group-wise N=512 bf16 GEMM staged two
    groups ahead into SBUF; per-step per-chain identity matmuls inject the
    z/r slices into the recurrence PSUM banks (biases folded in at GEMM
    evacuation); the hh slice joins via DVE (it sits outside the r* product).
  - Gate algebra per chain: zc = sigmoid(-z_pre) directly on ACT;
    GPSIMD computes a' = zc*h, s = h - a' (= z*h) and the bf16 state
    h_new = s + b, while DVE computes b = tanh_out*zc and the fp8 state
    h8 = s + b, so no cast sits on the critical cycle.
  - PSUM: 6 gate banks (z/r/hr per chain, single-buffered) + 2 GEMM banks.
"""

import os
import sys

sys.path.insert(0, "/opt/trn_rl_repo")

import numpy as np
import ml_dtypes

import concourse.bass as bass
import concourse.mybir as mybir
import concourse.tile as tile
from concourse import bacc
from concourse.bass import ds
from concourse.bass_utils import run_bass_kernel_spmd

BF16 = mybir.dt.bfloat16
F32 = mybir.dt.float32
FP8 = mybir.dt.float8e4
NP_BF16 = ml_dtypes.bfloat16
NP_FP8 = ml_dtypes.float8_e4m3

NCORES = 8
B, T, F, H = 512, 128, 512, 512
BS = B // NCORES          # 64 batch rows per core
SB = BS // 2              # 32 rows per chain (2 chains per core)
G3 = 3 * H                # 1536
KF = F // 128             # 4 contraction chunks for x @ kernel
KH = H // 128             # 4 contraction chunks for h @ rkernel
NM = G3 // 128            # 12 gate chunks of 128
GROUP = 8                 # timesteps per x-proj GEMM group
NB = GROUP * BS           # 512 free columns per GEMM group
HC = KH * SB              # 128: per-chain hidden free dim (chunk-major)
AF = mybir.ActivationFunctionType
ALU = mybir.AluOpType
DR = mybir.MatmulPerfMode.DoubleRow


def build_program(n_steps=T):
    """Emit the full Bass/Tile program for one core."""
    n_groups = (n_steps + GROUP - 1) // GROUP
    nc = bacc.Bacc()

    # ---- DRAM parameters (per-core shapes; host pre-arranges layouts) ----
    xT = nc.declare_dram_parameter("xT", [F, T * BS], BF16, isOutput=False)
    wk = nc.declare_dram_parameter("wk", [F, G3], BF16, isOutput=False)
    wr8 = nc.declare_dram_parameter("wr8", [H, G3], FP8, isOutput=False)
    ident = nc.declare_dram_parameter("ident", [128, 128], BF16, isOutput=False)
    # brh rows 0..3 hold bias_r[2H:] chunks; rows 4..127 zero (keeps the
    # (128,128) PE tile config).  ind rows j indicate cols [j*SB,(j+1)*SB).
    brh_l = nc.declare_dram_parameter("brh_l", [128, 128], BF16, isOutput=False)
    ind = nc.declare_dram_parameter("ind", [128, HC], BF16, isOutput=False)
    bias_cols = nc.declare_dram_parameter("bias_cols", [128, NM], F32, isOutput=False)
    wd = nc.declare_dram_parameter("wd", [128, KH], BF16, isOutput=False)
    db = nc.declare_dram_parameter("db", [1, 1], F32, isOutput=False)
    out = nc.declare_dram_parameter("out", [1, BS], F32, isOutput=True)

    xT_v = xT.ap().rearrange("(k p) n -> p k n", p=128)    # [128, KF, T*BS]
    wk_v = wk.ap().rearrange("(k p) g -> p k g", p=128)    # [128, KF, G3]
    wr8_v = wr8.ap().rearrange("(k p) g -> p k g", p=128)  # [128, KH, G3]

    with tile.TileContext(nc) as tc:
        with (
            tc.tile_pool(name="const", bufs=1) as p_const,
            tc.tile_pool(name="xt", bufs=4) as p_xt,
            tc.tile_pool(name="xp", bufs=3) as p_xp,
            tc.tile_pool(name="st", bufs=2) as p_st,
            tc.tile_pool(name="ew", bufs=2) as p_ew,
            tc.tile_pool(name="gate", bufs=1, space="PSUM") as p_gate,
            tc.tile_pool(name="gps", bufs=2, space="PSUM") as p_g,
        ):
            # ---- resident constants ----
            wk_sb = p_const.tile([128, KF, G3], BF16)
            nc.sync.dma_start(out=wk_sb[:, :, :], in_=wk_v)
            wr8_sb = p_const.tile([128, KH, G3], FP8)
            nc.sync.dma_start(out=wr8_sb[:, :, :], in_=wr8_v)
            ident_sb = p_const.tile([128, 128], BF16)
            nc.sync.dma_start(out=ident_sb[:, :], in_=ident.ap())
            brh_sb = p_const.tile([128, 128], BF16)
            nc.sync.dma_start(out=brh_sb[:, :], in_=brh_l.ap())
            ind_sb = p_const.tile([128, HC], BF16)
            nc.sync.dma_start(out=ind_sb[:, :], in_=ind.ap())
            bias_sb = p_const.tile([128, NM], F32)
            nc.sync.dma_start(out=bias_sb[:, :], in_=bias_cols.ap())
            wd_sb = p_const.tile([128, KH], BF16)
            nc.sync.dma_start(out=wd_sb[:, :], in_=wd.ap())
            db_sb = p_const.tile([1, 1], F32)
            nc.sync.dma_start(out=db_sb[:, :], in_=db.ap())

            # ---- xT group DMA + x-proj group GEMM (all 12 slices) ----
            xt_tiles = {}

            def emit_xt_dma(g):
                if g >= n_groups:
                    return
                t0 = p_xt.tile([128, KF, NB], BF16, name=f"xt{g}", tag="xt")
                nc.sync.dma_start(out=t0[:, :, :], in_=xT_v[:, :, ds(g * NB, NB)])
                xt_tiles[g] = t0

            xp_tiles = {}
            gemm_ps = {}

            def emit_xp_alloc(g):
                if g >= n_groups:
                    return
                xp_tiles[g] = p_xp.tile([128, NM, NB], BF16, name=f"xp{g}", tag="xp")

            def emit_gemm_mms(g, m):
                if g >= n_groups:
                    return
                ps = p_g.tile([128, NB], F32, name=f"gps{g}_{m}", tag="gps")
                gemm_ps[(g, m)] = ps
                xt_t = xt_tiles[g]
                for k in range(KF):
                    nc.tensor.matmul(
                        out=ps[:, :],
                        lhsT=wk_sb[:, k, ds(m * 128, 128)],
                        rhs=xt_t[:, k, :],
                        start=(k == 0),
                        stop=(k == KF - 1),
                    )

            def emit_gemm_evac(g, m, engine):
                if g >= n_groups or (g, m) not in gemm_ps:
                    return
                ps = gemm_ps.pop((g, m))
                dst = xp_tiles[g][:, m, :]
                b_ap = bias_sb[:, ds(m, 1)]
                if engine == "act":
                    nc.scalar.activation(dst, ps[:, :], AF.Identity, bias=b_ap)
                else:
                    nc.vector.tensor_scalar_add(out=dst, in0=ps[:, :], scalar1=b_ap)

            # ---- per-chain state ----
            h8 = [None, None]     # fp8 state (matmul input)
            hbf = [None, None]    # bf16 state (blend input)
            banks = {}            # (t, c) -> (z_ps, r_ps, hr_ps)

            def emit_inject(t, c):
                """Seed chain c's step-t PSUM banks: xp_z/xp_r via identity
                matmul (start=True), bias_r[2H:] into the hr bank."""
                if t >= n_steps:
                    return
                g, tau = divmod(t, GROUP)
                co = tau * BS + c * SB
                xp_g = xp_tiles[g]
                z_ps = p_gate.tile([128, HC], F32, name=f"z{t}_{c}", tag=f"z{c}")
                r_ps = p_gate.tile([128, HC], F32, name=f"r{t}_{c}", tag=f"r{c}")
                hr_ps = p_gate.tile([128, HC], F32, name=f"hr{t}_{c}", tag=f"hr{c}")
                nc.tensor.matmul(
                    out=z_ps[:, :], lhsT=ident_sb[:, :],
                    rhs=xp_g[:, 0:KH, ds(co, SB)], start=True, stop=False,
                )
                nc.tensor.matmul(
                    out=r_ps[:, :], lhsT=ident_sb[:, :],
                    rhs=xp_g[:, KH:2 * KH, ds(co, SB)], start=True, stop=False,
                )
                nc.tensor.matmul(
                    out=hr_ps[:, :], lhsT=brh_sb[:, :], rhs=ind_sb[:, :],
                    start=True, stop=False,
                )
                banks[(t, c)] = (z_ps, r_ps, hr_ps)

            def emit_rec(t, c):
                """fp8 DoubleRow recurrence matmuls for chain c, step t.
                Gate order r, hr, z so sigmoid(r) fires earliest."""
                z_ps, r_ps, hr_ps = banks[(t, c)]
                h8c = h8[c]
                for gate, ps in ((1, r_ps), (2, hr_ps), (0, z_ps)):
                    for m in range(KH):
                        for jp in range(2):
                            last = m == KH - 1 and jp == 1
                            rhs = h8c[:, ds(jp * 2 * SB, 2 * SB)].rearrange(
                                "p (two b) -> p two b", two=2
                            )
                            nc.tensor.matmul(
                                out=ps[:, ds(m * SB, SB)],
                                lhsT=wr8_sb[:, ds(2 * jp, 2), ds((gate * KH + m) * 128, 128)],
                                rhs=rhs,
                                start=False, stop=last,
                                skip_group_check=not last,
                                perf_mode=DR,
                            )

            def emit_chain(t, c, evac=None):
                """Elementwise gate math for chain c, step t.  A deferred
                x-proj evacuation (if any) is slotted into the engine's
                natural idle window: ACT between zc and tanh, DVE between t2
                and b — anywhere else it head-of-line-blocks the other
                chain's ops."""
                g, tau = divmod(t, GROUP)
                co = tau * BS + c * SB
                z_ps, r_ps, hr_ps = banks.pop((t, c))
                xp_g = xp_tiles[g]
                hbf_p = hbf[c]

                rsig = p_ew.tile([128, HC], BF16, name=f"rs{t}_{c}", tag=f"rsig{c}")
                nc.scalar.activation(rsig[:, :], r_ps[:, :], AF.Sigmoid)
                zc = p_ew.tile([128, HC], BF16, name=f"zc{t}_{c}", tag=f"zc{c}")
                nc.scalar.activation(zc[:, :], z_ps[:, :], AF.Sigmoid, scale=-1.0)
                if evac is not None and evac[2] == "act":
                    emit_gemm_evac(*evac)

                t1 = p_ew.tile([128, HC], F32, name=f"t1_{t}_{c}", tag=f"t1{c}")
                nc.vector.tensor_tensor(
                    out=t1[:, :], in0=hr_ps[:, :], in1=rsig[:, :], op=ALU.mult
                )
                t2 = p_ew.tile([128, KH, SB], F32, name=f"t2_{t}_{c}", tag=f"t2{c}")
                nc.vector.tensor_tensor(
                    out=t2[:, :, :],
                    in0=t1[:, :].rearrange("p (m b) -> p m b", b=SB),
                    in1=xp_g[:, 2 * KH:NM, ds(co, SB)],
                    op=ALU.add,
                )
                if evac is not None and evac[2] == "dve":
                    emit_gemm_evac(*evac)
                th = p_ew.tile([128, HC], BF16, name=f"th{t}_{c}", tag=f"th{c}")
                nc.scalar.activation(
                    th[:, :], t2[:, :, :].rearrange("p m b -> p (m b)"), AF.Tanh
                )

                # GPSIMD (off-cycle): a' = zc*h, s = h - a' (= z*h)
                ap_t = p_ew.tile([128, HC], BF16, name=f"ap{t}_{c}", tag=f"ap{c}")
                nc.gpsimd.tensor_mul(ap_t[:, :], zc[:, :], hbf_p[:, :])
                s_t = p_ew.tile([128, HC], BF16, name=f"s{t}_{c}", tag=f"s{c}")
                nc.gpsimd.tensor_tensor(
                    out=s_t[:, :], in0=hbf_p[:, :], in1=ap_t[:, :], op=ALU.subtract
                )

                # tail: b = tanh*zc [DVE]; h8 = s+b fp8 [DVE]; hbf = s+b [GPS]
                b_t = p_ew.tile([128, HC], BF16, name=f"b{t}_{c}", tag=f"b{c}")
                nc.vector.tensor_tensor(
                    out=b_t[:, :], in0=th[:, :], in1=zc[:, :], op=ALU.mult
                )
                h8_n = p_st.tile([128, HC], FP8, name=f"h8_{t}_{c}", tag=f"h8{c}")
                nc.vector.tensor_tensor(
                    out=h8_n[:, :], in0=s_t[:, :], in1=b_t[:, :], op=ALU.add
                )
                hbf_n = p_st.tile([128, HC], BF16, name=f"hb{t}_{c}", tag=f"hb{c}")
                nc.gpsimd.tensor_tensor(
                    out=hbf_n[:, :], in0=s_t[:, :], in1=b_t[:, :], op=ALU.add
                )
                h8[c] = h8_n
                hbf[c] = hbf_n

            # ---- prologue ----
            for g in range(min(3, n_groups)):
                emit_xt_dma(g)
            for g in range(min(2, n_groups)):
                emit_xp_alloc(g)
                for m in range(NM):
                    emit_gemm_mms(g, m)
                    emit_gemm_evac(g, m, "act" if m % 2 == 0 else "dve")
            for c in range(2):
                h8_0 = p_st.tile([128, HC], FP8, name=f"h8init{c}", tag=f"h8{c}")
                nc.vector.memset(h8_0[:, :], 0.0)
                hbf_0 = p_st.tile([128, HC], BF16, name=f"hbinit{c}", tag=f"hb{c}")
                nc.gpsimd.memset(hbf_0[:, :], 0.0)
                h8[c] = h8_0
                hbf[c] = hbf_0
                emit_inject(0, c)

            # ---- main loop: per step, chain A block then chain B block ----
            pending_evacs = []
            for t in range(n_steps):
                g, tau = divmod(t, GROUP)
                if tau == 0:
                    emit_xt_dma(g + 3)
                    emit_xp_alloc(g + 2)
                sl_lo = (tau * NM) // GROUP
                sl_hi = ((tau + 1) * NM) // GROUP
                slices = list(range(sl_lo, sl_hi))

                for c in range(2):
                    emit_rec(t, c)
                    evac = pending_evacs.pop(0) if pending_evacs else None
                    emit_chain(t, c, evac)
                    # amortized group GEMM: half the slices per chain block
                    my = slices[c::2] if len(slices) > 1 else (slices if c == 0 else [])
                    for m in my:
                        emit_gemm_mms(g + 2, m)
                        pending_evacs.append((g + 2, m, "dve" if m % 2 == 0 else "act"))
                    emit_inject(t + 1, c)
            for e in pending_evacs:
                emit_gemm_evac(*e)

            # ---- dense head: out = h_last @ dense_w + dense_b ----
            for c in range(2):
                d_ps = p_g.tile([1, SB], F32, name=f"dense{c}", tag="gps")
                for k in range(KH):
                    nc.tensor.matmul(
                        out=d_ps[0:1, :],
                        lhsT=wd_sb[:, ds(k, 1)],
                        rhs=hbf[c][:, ds(k * SB, SB)],
                        start=(k == 0),
                        stop=(k == KH - 1),
                    )
                out_sb = p_const.tile([1, SB], F32, name=f"out{c}")
                nc.scalar.activation(
                    out_sb[0:1, :], d_ps[0:1, :], AF.Identity, bias=db_sb[0:1, 0:1]
                )
                nc.sync.dma_start(out=out.ap()[:, ds(c * SB, SB)], in_=out_sb[0:1, :])

    nc.finalize()
    return nc


def prep_inputs(x, kernel, rkernel, bias_i, bias_r, dense_w, dense_b, n_steps=T):
    """Host-side shard + layout prep. Returns in_maps for run_bass_kernel_spmd."""
    x = np.asarray(x, dtype=np.float32)
    kernel = np.asarray(kernel, dtype=np.float32)
    rkernel = np.asarray(rkernel, dtype=np.float32)
    bias_i = np.asarray(bias_i, dtype=np.float32)
    bias_r = np.asarray(bias_r, dtype=np.float32)
    dense_w = np.asarray(dense_w, dtype=np.float32)
    dense_b = np.asarray(dense_b, dtype=np.float32)

    wk_h = np.ascontiguousarray(kernel.astype(NP_BF16))
    wr8_h = np.ascontiguousarray(rkernel.astype(NP_FP8))
    ident_h = np.eye(128, dtype=NP_BF16)
    brh_h = np.zeros((128, 128), dtype=NP_BF16)
    brh_h[:KH] = bias_r[2 * H:].reshape(KH, 128).astype(NP_BF16)
    ind_h = np.zeros((128, HC), dtype=NP_BF16)
    for j in range(KH):
        ind_h[j, j * SB:(j + 1) * SB] = 1
    comb = np.concatenate([bias_i[: 2 * H] + bias_r[: 2 * H], bias_i[2 * H:]])
    bias_cols_h = np.ascontiguousarray(comb.reshape(NM, 128).T.astype(np.float32))
    wd_h = np.ascontiguousarray(dense_w.reshape(KH, 128).T.astype(NP_BF16))
    db_h = dense_b.reshape(1, 1).astype(np.float32)

    in_maps = []
    for c in range(NCORES):
        xs = x[c * BS:(c + 1) * BS]                       # [BS, T, F]
        xT_h = np.ascontiguousarray(
            xs.transpose(2, 1, 0).reshape(F, T * BS).astype(NP_BF16)
        )
        in_maps.append(
            {
                "xT": xT_h,
                "wk": wk_h,
                "wr8": wr8_h,
                "ident": ident_h,
                "brh_l": brh_h,
                "ind": ind_h,
                "bias_cols": bias_cols_h,
                "wd": wd_h,
                "db": db_h,
            }
        )
    return in_maps


def kernel(x, kernel, rkernel, bias_i, bias_r, dense_w, dense_b):
    nc = build_program()
    in_maps = prep_inputs(x, kernel, rkernel, bias_i, bias_r, dense_w, dense_b)
    res = run_bass_kernel_spmd(nc, in_maps, list(range(NCORES)))
    outs = [res.results[i]["out"].reshape(BS, 1) for i in range(NCORES)]
    return np.concatenate(outs, axis=0).astype(np.float32)


# revision 28
# speedup vs baseline: 1.5136x; 1.5136x over previous
"""Trainium2 Bass kernel for a Keras-style GRU (reset_after=True) + Dense(1) head.

Reference computation (per batch row):
    x_proj = x @ kernel + bias_i                      # [T, 3H]
    per step t:  hp = h @ rkernel + bias_r            # [3H]
        z  = sigmoid(xp[:H]      + hp[:H])
        r  = sigmoid(xp[H:2H]    + hp[H:2H])
        hh = tanh   (xp[2H:]     + r * hp[2H:])
        h  = z * h + (1 - z) * hh
    out = h_last @ dense_w + dense_b                  # [1]

Strategy (8 NeuronCores, data-parallel over batch, 64 rows/core):
  - The per-step serial cycle (rec matmuls -> sigmoid -> t1 -> t2 -> tanh ->
    blend -> next rec) is latency-bound by ~5 cross-engine semaphore hops of
    ~300ns each, so a single dependence chain can't go below ~3.5us/step.
    Instead each core runs TWO independent GRU chains (batch rows split
    32+32), phase-shifted half a cycle: every ~3.5us cycle completes one step
    of BOTH chains => ~1.75us/step effective.
  - All recurrence matmuls are fp8(e4m3) DoubleRow (two K-chunks per
    instruction, half cycles/col): h is quantized to fp8 as a matmul input
    only; the carried state stays bf16 (numerically validated: ~1e-2).
  - x_proj for all three gates is a group-wise N=512 bf16 GEMM staged two
    groups ahead into SBUF; per-step per-chain identity matmuls inject the
    z/r slices into the recurrence PSUM banks (biases folded in at GEMM
    evacuation); the hh slice joins via DVE (it sits outside the r* product).
  - Gate algebra per chain: zc = sigmoid(-z_pre) directly on ACT;
    GPSIMD computes a' = zc*h, s = h - a' (= z*h) and the bf16 state
    h_new = s + b, while DVE computes b = tanh_out*zc and the fp8 state
    h8 = s + b, so no cast sits on the critical cycle.
  - PSUM: 6 gate banks (z/r/hr per chain, single-buffered) + 2 GEMM banks.
"""

import os
import sys

sys.path.insert(0, "/opt/trn_rl_repo")

import numpy as np
import ml_dtypes

import concourse.bass as bass
import concourse.mybir as mybir
import concourse.tile as tile
from concourse import bacc
from concourse.bass import ds
from concourse.bass_utils import run_bass_kernel_spmd

BF16 = mybir.dt.bfloat16
F32 = mybir.dt.float32
FP8 = mybir.dt.float8e4
NP_BF16 = ml_dtypes.bfloat16
NP_FP8 = ml_dtypes.float8_e4m3

NCORES = 8
B, T, F, H = 512, 128, 512, 512
BS = B // NCORES          # 64 batch rows per core
SB = BS // 2              # 32 rows per chain (2 chains per core)
G3 = 3 * H                # 1536
KF = F // 128             # 4 contraction chunks for x @ kernel
KH = H // 128             # 4 contraction chunks for h @ rkernel
NM = G3 // 128            # 12 gate chunks of 128
GROUP = 8                 # timesteps per x-proj GEMM group
NB = GROUP * BS           # 512 free columns per GEMM group
HC = KH * SB              # 128: per-chain hidden free dim (chunk-major)
AF = mybir.ActivationFunctionType
ALU = mybir.AluOpType
DR = mybir.MatmulPerfMode.DoubleRow


def build_program(n_steps=T):
    """Emit the full Bass/Tile program for one core."""
    n_groups = (n_steps + GROUP - 1) // GROUP
    nc = bacc.Bacc()

    # ---- DRAM parameters (per-core shapes; host pre-arranges layouts) ----
    xT = nc.declare_dram_parameter("xT", [F, T * BS], BF16, isOutput=False)
    wk = nc.declare_dram_parameter("wk", [F, G3], BF16, isOutput=False)
    wr = nc.declare_dram_parameter("wr", [H, G3], BF16, isOutput=False)
    ident = nc.declare_dram_parameter("ident", [128, 128], BF16, isOutput=False)
    # brh rows 0..3 hold bias_r[2H:] chunks; rows 4..127 zero (keeps the
    # (128,128) PE tile config).  ind rows j indicate cols [j*SB,(j+1)*SB).
    brh_l = nc.declare_dram_parameter("brh_l", [128, 128], BF16, isOutput=False)
    ind = nc.declare_dram_parameter("ind", [128, HC], BF16, isOutput=False)
    bias_cols = nc.declare_dram_parameter("bias_cols", [128, NM], F32, isOutput=False)
    wd = nc.declare_dram_parameter("wd", [128, KH], BF16, isOutput=False)
    db = nc.declare_dram_parameter("db", [1, 1], F32, isOutput=False)
    out = nc.declare_dram_parameter("out", [1, BS], F32, isOutput=True)

    xT_v = xT.ap().rearrange("(k p) n -> p k n", p=128)    # [128, KF, T*BS]
    wk_v = wk.ap().rearrange("(k p) g -> p k g", p=128)    # [128, KF, G3]
    wr_v = wr.ap().rearrange("(k p) g -> p k g", p=128)    # [128, KH, G3]

    with tile.TileContext(nc) as tc:
        with (
            tc.tile_pool(name="const", bufs=1) as p_const,
            tc.tile_pool(name="xt", bufs=4) as p_xt,
            tc.tile_pool(name="xp", bufs=3) as p_xp,
            tc.tile_pool(name="st", bufs=2) as p_st,
            tc.tile_pool(name="ew", bufs=2) as p_ew,
            tc.tile_pool(name="gate", bufs=1, space="PSUM") as p_gate,
            tc.tile_pool(name="gps", bufs=2, space="PSUM") as p_g,
        ):
            # ---- resident constants ----
            wk_sb = p_const.tile([128, KF, G3], BF16)
            nc.sync.dma_start(out=wk_sb[:, :, :], in_=wk_v)
            wr_sb = p_const.tile([128, KH, G3], BF16)
            nc.sync.dma_start(out=wr_sb[:, :, :], in_=wr_v)
            ident_sb = p_const.tile([128, 128], BF16)
            nc.sync.dma_start(out=ident_sb[:, :], in_=ident.ap())
            brh_sb = p_const.tile([128, 128], BF16)
            nc.sync.dma_start(out=brh_sb[:, :], in_=brh_l.ap())
            ind_sb = p_const.tile([128, HC], BF16)
            nc.sync.dma_start(out=ind_sb[:, :], in_=ind.ap())
            bias_sb = p_const.tile([128, NM], F32)
            nc.sync.dma_start(out=bias_sb[:, :], in_=bias_cols.ap())
            wd_sb = p_const.tile([128, KH], BF16)
            nc.sync.dma_start(out=wd_sb[:, :], in_=wd.ap())
            db_sb = p_const.tile([1, 1], F32)
            nc.sync.dma_start(out=db_sb[:, :], in_=db.ap())

            # ---- xT group DMA + x-proj group GEMM (all 12 slices) ----
            xt_tiles = {}

            def emit_xt_dma(g):
                if g >= n_groups:
                    return
                t0 = p_xt.tile([128, KF, NB], BF16, name=f"xt{g}", tag="xt")
                nc.sync.dma_start(out=t0[:, :, :], in_=xT_v[:, :, ds(g * NB, NB)])
                xt_tiles[g] = t0

            xp_tiles = {}
            gemm_ps = {}

            def emit_xp_alloc(g):
                if g >= n_groups:
                    return
                xp_tiles[g] = p_xp.tile([128, NM, NB], BF16, name=f"xp{g}", tag="xp")

            def emit_gemm_mms(g, m):
                if g >= n_groups:
                    return
                ps = p_g.tile([128, NB], F32, name=f"gps{g}_{m}", tag="gps")
                gemm_ps[(g, m)] = ps
                xt_t = xt_tiles[g]
                for k in range(KF):
                    nc.tensor.matmul(
                        out=ps[:, :],
                        lhsT=wk_sb[:, k, ds(m * 128, 128)],
                        rhs=xt_t[:, k, :],
                        start=(k == 0),
                        stop=(k == KF - 1),
                    )

            def emit_gemm_evac(g, m, engine):
                if g >= n_groups or (g, m) not in gemm_ps:
                    return
                ps = gemm_ps.pop((g, m))
                dst = xp_tiles[g][:, m, :]
                b_ap = bias_sb[:, ds(m, 1)]
                if engine == "act":
                    nc.scalar.activation(dst, ps[:, :], AF.Identity, bias=b_ap)
                else:
                    nc.vector.tensor_scalar_add(out=dst, in0=ps[:, :], scalar1=b_ap)

            # ---- per-chain state ----
            hst = [None, None]    # bf16 hidden state per chain
            banks = {}            # (t, c) -> (z_ps, r_ps, hr_ps)

            def emit_inject(t, c):
                """Seed chain c's step-t PSUM banks: xp_z/xp_r via identity
                matmul (start=True), bias_r[2H:] into the hr bank."""
                if t >= n_steps:
                    return
                g, tau = divmod(t, GROUP)
                co = tau * BS + c * SB
                xp_g = xp_tiles[g]
                z_ps = p_gate.tile([128, HC], F32, name=f"z{t}_{c}", tag=f"z{c}")
                r_ps = p_gate.tile([128, HC], F32, name=f"r{t}_{c}", tag=f"r{c}")
                hr_ps = p_gate.tile([128, HC], F32, name=f"hr{t}_{c}", tag=f"hr{c}")
                nc.tensor.matmul(
                    out=z_ps[:, :], lhsT=ident_sb[:, :],
                    rhs=xp_g[:, 0:KH, ds(co, SB)], start=True, stop=False,
                )
                nc.tensor.matmul(
                    out=r_ps[:, :], lhsT=ident_sb[:, :],
                    rhs=xp_g[:, KH:2 * KH, ds(co, SB)], start=True, stop=False,
                )
                nc.tensor.matmul(
                    out=hr_ps[:, :], lhsT=brh_sb[:, :], rhs=ind_sb[:, :],
                    start=True, stop=False,
                )
                banks[(t, c)] = (z_ps, r_ps, hr_ps)

            def emit_rec(t, c):
                """Recurrence matmuls for chain c, step t.  Gate order
                r, hr, z so sigmoid(r) fires earliest."""
                z_ps, r_ps, hr_ps = banks[(t, c)]
                hc = hst[c]
                for gate, ps in ((1, r_ps), (2, hr_ps), (0, z_ps)):
                    for m in range(KH):
                        for k in range(KH):
                            last = m == KH - 1 and k == KH - 1
                            nc.tensor.matmul(
                                out=ps[:, ds(m * SB, SB)],
                                lhsT=wr_sb[:, k, ds((gate * KH + m) * 128, 128)],
                                rhs=hc[:, ds(k * SB, SB)],
                                start=False, stop=last,
                                skip_group_check=not last,
                            )

            def emit_chain(t, c, evac=None):
                """Elementwise gate math for chain c, step t.  A deferred
                x-proj evacuation (if any) is slotted into the engine's
                natural idle window: ACT between zc and tanh, DVE between t2
                and b — anywhere else it head-of-line-blocks the other
                chain's ops."""
                g, tau = divmod(t, GROUP)
                co = tau * BS + c * SB
                z_ps, r_ps, hr_ps = banks.pop((t, c))
                xp_g = xp_tiles[g]
                h_p = hst[c]

                rsig = p_ew.tile([128, HC], BF16, name=f"rs{t}_{c}", tag=f"rsig{c}")
                nc.scalar.activation(rsig[:, :], r_ps[:, :], AF.Sigmoid)
                zc = p_ew.tile([128, HC], BF16, name=f"zc{t}_{c}", tag=f"zc{c}")
                nc.scalar.activation(zc[:, :], z_ps[:, :], AF.Sigmoid, scale=-1.0)
                if evac is not None and evac[2] == "act":
                    emit_gemm_evac(*evac)

                t1 = p_ew.tile([128, HC], F32, name=f"t1_{t}_{c}", tag=f"t1{c}")
                nc.vector.tensor_tensor(
                    out=t1[:, :], in0=hr_ps[:, :], in1=rsig[:, :], op=ALU.mult
                )
                t2 = p_ew.tile([128, KH, SB], F32, name=f"t2_{t}_{c}", tag=f"t2{c}")
                nc.vector.tensor_tensor(
                    out=t2[:, :, :],
                    in0=t1[:, :].rearrange("p (m b) -> p m b", b=SB),
                    in1=xp_g[:, 2 * KH:NM, ds(co, SB)],
                    op=ALU.add,
                )
                if evac is not None and evac[2] == "dve":
                    emit_gemm_evac(*evac)
                th = p_ew.tile([128, HC], BF16, name=f"th{t}_{c}", tag=f"th{c}")
                nc.scalar.activation(
                    th[:, :], t2[:, :, :].rearrange("p m b -> p (m b)"), AF.Tanh
                )

                # GPSIMD (off-cycle): a' = zc*h, s = h - a' (= z*h)
                ap_t = p_ew.tile([128, HC], BF16, name=f"ap{t}_{c}", tag=f"ap{c}")
                nc.gpsimd.tensor_mul(ap_t[:, :], zc[:, :], h_p[:, :])
                s_t = p_ew.tile([128, HC], BF16, name=f"s{t}_{c}", tag=f"s{c}")
                nc.gpsimd.tensor_tensor(
                    out=s_t[:, :], in0=h_p[:, :], in1=ap_t[:, :], op=ALU.subtract
                )

                # tail: b = tanh*zc [DVE]; h_new = s+b [DVE]
                b_t = p_ew.tile([128, HC], BF16, name=f"b{t}_{c}", tag=f"b{c}")
                nc.vector.tensor_tensor(
                    out=b_t[:, :], in0=th[:, :], in1=zc[:, :], op=ALU.mult
                )
                h_n = p_st.tile([128, HC], BF16, name=f"h{t}_{c}", tag=f"h{c}")
                nc.vector.tensor_tensor(
                    out=h_n[:, :], in0=s_t[:, :], in1=b_t[:, :], op=ALU.add
                )
                hst[c] = h_n

            # ---- prologue ----
            for g in range(min(3, n_groups)):
                emit_xt_dma(g)
            for g in range(min(2, n_groups)):
                emit_xp_alloc(g)
                for m in range(NM):
                    emit_gemm_mms(g, m)
                    emit_gemm_evac(g, m, "act" if m % 2 == 0 else "dve")
            for c in range(2):
                h_0 = p_st.tile([128, HC], BF16, name=f"hinit{c}", tag=f"h{c}")
                nc.vector.memset(h_0[:, :], 0.0)
                hst[c] = h_0
                emit_inject(0, c)

            # ---- main loop: per step, chain A block then chain B block ----
            pending_evacs = []
            for t in range(n_steps):
                g, tau = divmod(t, GROUP)
                if tau == 0:
                    emit_xt_dma(g + 3)
                    emit_xp_alloc(g + 2)
                sl_lo = (tau * NM) // GROUP
                sl_hi = ((tau + 1) * NM) // GROUP
                slices = list(range(sl_lo, sl_hi))

                for c in range(2):
                    emit_rec(t, c)
                    evac = pending_evacs.pop(0) if pending_evacs else None
                    emit_chain(t, c, evac)
                    # amortized group GEMM: half the slices per chain block
                    my = slices[c::2] if len(slices) > 1 else (slices if c == 0 else [])
                    for m in my:
                        emit_gemm_mms(g + 2, m)
                        pending_evacs.append((g + 2, m, "dve" if m % 2 == 0 else "act"))
                    emit_inject(t + 1, c)
            for e in pending_evacs:
                emit_gemm_evac(*e)

            # ---- dense head: out = h_last @ dense_w + dense_b ----
            for c in range(2):
                d_ps = p_g.tile([1, SB], F32, name=f"dense{c}", tag="gps")
                for k in range(KH):
                    nc.tensor.matmul(
                        out=d_ps[0:1, :],
                        lhsT=wd_sb[:, ds(k, 1)],
                        rhs=hst[c][:, ds(k * SB, SB)],
                        start=(k == 0),
                        stop=(k == KH - 1),
                    )
                out_sb = p_const.tile([1, SB], F32, name=f"out{c}")
                nc.scalar.activation(
                    out_sb[0:1, :], d_ps[0:1, :], AF.Identity, bias=db_sb[0:1, 0:1]
                )
                nc.sync.dma_start(out=out.ap()[:, ds(c * SB, SB)], in_=out_sb[0:1, :])

    nc.finalize()
    return nc


def prep_inputs(x, kernel, rkernel, bias_i, bias_r, dense_w, dense_b, n_steps=T):
    """Host-side shard + layout prep. Returns in_maps for run_bass_kernel_spmd."""
    x = np.asarray(x, dtype=np.float32)
    kernel = np.asarray(kernel, dtype=np.float32)
    rkernel = np.asarray(rkernel, dtype=np.float32)
    bias_i = np.asarray(bias_i, dtype=np.float32)
    bias_r = np.asarray(bias_r, dtype=np.float32)
    dense_w = np.asarray(dense_w, dtype=np.float32)
    dense_b = np.asarray(dense_b, dtype=np.float32)

    wk_h = np.ascontiguousarray(kernel.astype(NP_BF16))
    wr_h = np.ascontiguousarray(rkernel.astype(NP_BF16))
    ident_h = np.eye(128, dtype=NP_BF16)
    brh_h = np.zeros((128, 128), dtype=NP_BF16)
    brh_h[:KH] = bias_r[2 * H:].reshape(KH, 128).astype(NP_BF16)
    ind_h = np.zeros((128, HC), dtype=NP_BF16)
    for j in range(KH):
        ind_h[j, j * SB:(j + 1) * SB] = 1
    comb = np.concatenate([bias_i[: 2 * H] + bias_r[: 2 * H], bias_i[2 * H:]])
    bias_cols_h = np.ascontiguousarray(comb.reshape(NM, 128).T.astype(np.float32))
    wd_h = np.ascontiguousarray(dense_w.reshape(KH, 128).T.astype(NP_BF16))
    db_h = dense_b.reshape(1, 1).astype(np.float32)

    in_maps = []
    for c in range(NCORES):
        xs = x[c * BS:(c + 1) * BS]                       # [BS, T, F]
        xT_h = np.ascontiguousarray(
            xs.transpose(2, 1, 0).reshape(F, T * BS).astype(NP_BF16)
        )
        in_maps.append(
            {
                "xT": xT_h,
                "wk": wk_h,
                "wr": wr_h,
                "ident": ident_h,
                "brh_l": brh_h,
                "ind": ind_h,
                "bias_cols": bias_cols_h,
                "wd": wd_h,
                "db": db_h,
            }
        )
    return in_maps


def kernel(x, kernel, rkernel, bias_i, bias_r, dense_w, dense_b):
    nc = build_program()
    in_maps = prep_inputs(x, kernel, rkernel, bias_i, bias_r, dense_w, dense_b)
    res = run_bass_kernel_spmd(nc, in_maps, list(range(NCORES)))
    outs = [res.results[i]["out"].reshape(BS, 1) for i in range(NCORES)]
    return np.concatenate(outs, axis=0).astype(np.float32)
